# revision 4
# baseline (speedup 1.0000x reference)
"""Trainium2 Bass kernel for nn_BranchValueHead (segment_reduce).

Full inputs in, full output out. Internally: data-parallel across 8
NeuronCores at graph boundaries (32 whole graphs per core, batch is
sorted). Per core the segment-sum over nodes is done as a stream of
one-hot matmuls on the PE accumulating into PSUM (transposed layout
[C, slots]), followed by the tiny MLP + masked per-graph reduction,
all on device. The host only does index prep, padding and layout.
"""

import numpy as np

# Problem dims (hardcoded per contract)
N = 2_000_000
C = 128
B = 256
K = 32
NEG_SLOPE = 0.01

NCORES = 8
GPC = B // NCORES  # graphs per core = 32
J = 64             # 128-node tiles per graph (graph padded to J*128 = 8192 nodes)
T = GPC * J        # tiles per core = 2048
S = GPC * K        # branch slots per core = 1024
BLK = 16           # tiles per DMA block (1 MiB per dma_start)

_CACHE = {}


def build_program(gpc=GPC, j=J, k=K, c=C, blk=BLK):
    """Build the per-core Bass program (SPMD: same program on all cores)."""
    import concourse.bacc as bacc
    import concourse.tile as tile
    from concourse import mybir

    f32 = mybir.dt.float32
    t_tiles = gpc * j
    s_slots = gpc * k

    nc = bacc.Bacc("TRN2", target_bir_lowering=False)

    emb = nc.dram_tensor("emb", [128, t_tiles * c], f32, kind="ExternalInput")
    slotc = nc.dram_tensor("slotc", [128, t_tiles], f32, kind="ExternalInput")
    iota = nc.dram_tensor("iota", [128, k], f32, kind="ExternalInput")
    w1 = nc.dram_tensor("w1", [c, c], f32, kind="ExternalInput")
    b1 = nc.dram_tensor("b1", [c, 1], f32, kind="ExternalInput")
    w2 = nc.dram_tensor("w2", [c, 1], f32, kind="ExternalInput")
    b2 = nc.dram_tensor("b2", [1, 1], f32, kind="ExternalInput")
    mask = nc.dram_tensor("mask", [1, s_slots], f32, kind="ExternalInput")
    gv = nc.dram_tensor("gv", [1, gpc], f32, kind="ExternalOutput")

    with tile.TileContext(nc) as tc:
        with (
            tc.tile_pool(name="consts", bufs=1) as consts,
            tc.tile_pool(name="embp", bufs=4) as embp,
            tc.tile_pool(name="ohp", bufs=8) as ohp,
            tc.tile_pool(name="mlp", bufs=1) as mlp,
            tc.tile_pool(name="gacc", bufs=4, space="PSUM") as gacc,
            tc.tile_pool(name="psmlp", bufs=1, space="PSUM") as psmlp,
        ):
            iota_sb = consts.tile([128, k], f32)
            nc.sync.dma_start(iota_sb[:], iota[:])
            slot_sb = consts.tile([128, t_tiles], f32)
            nc.sync.dma_start(slot_sb[:], slotc[:])
            w1_sb = consts.tile([c, c], f32)
            nc.sync.dma_start(w1_sb[:], w1[:])
            b1_sb = consts.tile([c, 1], f32)
            nc.sync.dma_start(b1_sb[:], b1[:])
            w2_sb = consts.tile([c, 1], f32)
            nc.sync.dma_start(w2_sb[:], w2[:])
            b2_sb = consts.tile([1, 1], f32)
            nc.sync.dma_start(b2_sb[:], b2[:])
            mask_sb = consts.tile([1, s_slots], f32)
            nc.sync.dma_start(mask_sb[:], mask[:])

            bemb_sb = mlp.tile([c, s_slots], f32)

            # Segment-sum: stream embed tiles, one-hot matmul-accumulate per
            # graph into a fresh PSUM bank; copy each finished graph to SBUF.
            g_ps = None
            for blki in range(t_tiles // blk):
                et = embp.tile([128, blk * c], f32)
                nc.sync.dma_start(
                    et[:], emb[:, blki * blk * c : (blki + 1) * blk * c]
                )
                for bi in range(blk):
                    t = blki * blk + bi
                    g = t // j
                    if t % j == 0:
                        g_ps = gacc.tile([c, k], f32)
                    oh = ohp.tile([128, k], f32)
                    nc.vector.tensor_tensor(
                        oh[:],
                        iota_sb[:],
                        slot_sb[:, t : t + 1].to_broadcast([128, k]),
                        mybir.AluOpType.is_equal,
                    )
                    nc.tensor.matmul(
                        g_ps[:],
                        lhsT=et[:, bi * c : (bi + 1) * c],
                        rhs=oh[:],
                        start=(t % j == 0),
                        stop=(t % j == j - 1),
                    )
                    if t % j == j - 1:
                        nc.scalar.activation(
                            bemb_sb[:, g * k : (g + 1) * k],
                            g_ps[:],
                            mybir.ActivationFunctionType.Copy,
                        )

            # MLP: h = lrelu(bemb @ W1 + b1) ; bv = h @ W2 + b2 (transposed)
            h_ps = psmlp.tile([c, s_slots], f32)
            for s0 in range(0, s_slots, 512):
                w = min(512, s_slots - s0)
                nc.tensor.matmul(
                    h_ps[:, s0 : s0 + w],
                    lhsT=w1_sb[:],
                    rhs=bemb_sb[:, s0 : s0 + w],
                    start=True,
                    stop=True,
                )
            hb_sb = mlp.tile([c, s_slots], f32)
            nc.scalar.activation(
                hb_sb[:],
                h_ps[:],
                mybir.ActivationFunctionType.Identity,
                bias=b1_sb[:],
            )
            hs_sb = mlp.tile([c, s_slots], f32)
            nc.vector.tensor_scalar(
                hs_sb[:], hb_sb[:], float(NEG_SLOPE), None, mybir.AluOpType.mult
            )
            hl_sb = mlp.tile([c, s_slots], f32)
            nc.vector.tensor_tensor(hl_sb[:], hb_sb[:], hs_sb[:], mybir.AluOpType.max)

            bv_ps = psmlp.tile([1, s_slots], f32)
            for s0 in range(0, s_slots, 512):
                w = min(512, s_slots - s0)
                nc.tensor.matmul(
                    bv_ps[:, s0 : s0 + w],
                    lhsT=w2_sb[:],
                    rhs=hl_sb[:, s0 : s0 + w],
                    start=True,
                    stop=True,
                )
            bv_sb = mlp.tile([1, s_slots], f32)
            nc.vector.tensor_scalar(
                bv_sb[:], bv_ps[:], b2_sb[0:1, 0:1], None, mybir.AluOpType.add
            )
            bvm_sb = mlp.tile([1, s_slots], f32)
            nc.vector.tensor_tensor(bvm_sb[:], bv_sb[:], mask_sb[:], mybir.AluOpType.mult)
            gv_sb = mlp.tile([1, gpc], f32)
            nc.vector.tensor_reduce(
                gv_sb[:],
                bvm_sb[:].rearrange("p (g k) -> p g k", k=k),
                axis=mybir.AxisListType.X,
                op=mybir.AluOpType.add,
            )
            nc.sync.dma_start(gv[:], gv_sb[:])

    nc.finalize()
    return nc


def host_prep(node_embed, batch, branch, W1, b1, W2, b2):
    """Shard + pad + lay out inputs per core. Index/layout work only."""
    node_embed = np.ascontiguousarray(np.asarray(node_embed, dtype=np.float32))
    batch = np.asarray(batch).astype(np.int64)
    branch = np.asarray(branch).astype(np.int64)
    W1 = np.ascontiguousarray(np.asarray(W1, dtype=np.float32)).reshape(C, C)
    b1v = np.asarray(b1, dtype=np.float32).reshape(C, 1)
    W2 = np.ascontiguousarray(np.asarray(W2, dtype=np.float32)).reshape(C, 1)
    b2v = np.asarray(b2, dtype=np.float32).reshape(1, 1)

    starts = np.searchsorted(batch, np.arange(B + 1))
    sizes = np.diff(starts)
    assert sizes.max() <= J * 128, f"graph too large: {sizes.max()} > {J * 128}"

    max_b = np.maximum.reduceat(branch, starts[:-1])
    max_b = np.where(sizes > 0, max_b, -1)
    mask_full = (np.arange(K)[None, :] <= max_b[:, None]).astype(np.float32)  # [B, K]

    iota = np.ascontiguousarray(
        np.broadcast_to(np.arange(K, dtype=np.float32), (128, K))
    )

    in_maps = []
    for core in range(NCORES):
        g0 = core * GPC
        pad = np.zeros((T * 128, C), np.float32)
        slot = np.full((T * 128,), float(K), np.float32)
        for gi in range(GPC):
            g = g0 + gi
            s, e = starts[g], starts[g + 1]
            n = e - s
            ofs = gi * J * 128
            pad[ofs : ofs + n] = node_embed[s:e]
            slot[ofs : ofs + n] = branch[s:e].astype(np.float32)
        emb2 = np.ascontiguousarray(
            pad.reshape(T, 128, C).transpose(1, 0, 2).reshape(128, T * C)
        )
        slotc = np.ascontiguousarray(slot.reshape(T, 128).T)
        mask_core = np.ascontiguousarray(
            mask_full[g0 : g0 + GPC].reshape(1, S)
        )
        in_maps.append(
            {
                "emb": emb2,
                "slotc": slotc,
                "iota": iota,
                "w1": W1,
                "b1": b1v,
                "w2": W2,
                "b2": b2v,
                "mask": mask_core,
            }
        )
    return in_maps


def _get_program():
    if "nc" not in _CACHE:
        _CACHE["nc"] = build_program()
    return _CACHE["nc"]


def run_on_device(in_maps, trace=False):
    from concourse.bass_utils import run_bass_kernel_spmd

    nc = _get_program()
    return run_bass_kernel_spmd(
        nc, in_maps, core_ids=list(range(NCORES)), trace=trace
    )


def kernel(**inputs) -> np.ndarray:
    in_maps = host_prep(
        inputs["node_embed"],
        inputs["batch"],
        inputs["branch"],
        inputs["W1"],
        inputs["b1"],
        inputs["W2"],
        inputs["b2"],
    )
    res = run_on_device(in_maps, trace=False)
    gv = np.concatenate([r["gv"] for r in res.results], axis=1)  # [1, B]
    return gv.reshape(B, 1).astype(np.float32)
